# revision 67
# baseline (speedup 1.0000x reference)
"""MiniBatchDiscrimination Trainium2 kernel (hw-sharded, symmetric bands).

reference:
    M = einsum('nhwf,fbc->nhwbc', x, T)          # [N,H,W,B,C]
    norm = sum_c |M[i] - M[j]|                   # [N,N,H,W,B]
    o_b  = sum_j exp(-norm)                      # [N,H,W,B]
    out  = concat([x, o_b], axis=3)              # [N,H,W,F+B]

Sharding: the pairwise block is over n only, so hw (=h*16+w, 256 values) is
embarrassingly parallel: core k owns hw in [32k, 32k+32) and computes the
FULL 32x32 pairwise block for its slice. No redundant M compute, 8x less
input DMA than data-parallel-over-n.

Symmetry: norm[i,j] == norm[j,i]. Rows are processed in 4 bands g (i in
[8g,8g+8)); row i computes STRICTLY j >= i (0.516x of the full block — the
symmetric minimum incl. diagonal). Skipped sub-diagonal cells of a band
read 0 from PSUM (the s=0 stripe's start=True zeroes them), so E holds
exp(0)=1 there — constants the host subtracts. Then
  o_b[n], n in group h = column sums over i<=n of E[q,g'] for g' <= h
    (PE stripe-matmuls with per-q ones matrices; + (7-jj) fakes)
  + full-band row sum over j of E[q,h] at row n (Pool add-trees, g=3 as a
    DVE reduce in the tail; + s fakes + a double-counted diagonal).
The two parts live in different layouts and are added on the host, which
also subtracts the constant fake terms.

Per-core device layout: M2[q] [(b16,c8) part, (n32,hw32) free] fp16 so that
  - M-compute is a plain matmul (lhsT = T-tile [f,(b,c)], rhs = xT [f,(n,hw)])
  - |M_j - M_i| is one custom DVE op (2X_1PORT, 2 elem/cycle) per (q, band,
    row) over the contiguous (j,hw) range, broadcasting the i-row via a
    0-stride AP dim
  - the c-reduction contracts the partition axis on the TensorEngine with
    stripe-ones matrices, 8 i-stripes accumulating into one PSUM tile
    [(i8,b16) part, (j,hw) free]
  - exp(-norm) is one ACT pass per (q, band).
Engines in steady state: DVE absdiff (the critical path), PE stripes +
column parts, ACT M2-copies + exp, Pool row-sum trees + 2 input DMAs.
A few zero matmuls at t=0 ramp the PE out of its low-clock pstate while
the (4-queue-parallel) input DMAs land.
"""

import os
import sys

for _p in ("/opt/trn_rl_repo", "/opt/pypackages"):
    if _p not in sys.path and os.path.isdir(_p):
        sys.path.append(_p)

import numpy as np

N, F, B, C = 32, 256, 64, 8
HW = 256        # total hw positions
HWL = 32        # hw positions per core
CORES = 8
FH = 2          # f in two partition halves of 128
Q = 4           # b-quarters of 16
G = 4           # i-bands of 8
GS = 8          # band size
F16 = "float16"

PERF_MAX = 1
# "pool": row sums as gpsimd add-trees; "dve": as DVE tensor_reduce.
# (A/B knob: CoreSim models Q7 launch at ~95ns; if real-hw gpsimd ops are
# slower, "dve" trades ~10us of modeled DVE time for no Pool ops at all.)
TAIL_ENGINE = "pool"


def _absdiff_uop_1x():
    """REGULAR program: |a-b| via SUB, reverse-SUB, MAX on slices 0-2."""
    from concourse.dve_uop import (
        ENABLE, AluInp, AluOp, DelayInp, InpSel, OutPath, OutSel, Trigger,
        UopConfig, UopDpConfig,
    )

    u = UopConfig()
    u.enable_input(InpSel.SRC_0, 0).enable_input(InpSel.SRC_1, 1)
    u.require_inp0 = ENABLE
    u.require_inp1 = ENABLE
    u.trigger = (Trigger.SRC_TENSOR_DONE, Trigger.NONE, Trigger.NONE)
    u.enable_output(OutSel.ALU_OUT, OutPath.WR0_LO)
    dp = u.datapath_config
    # s0: alu = a - b; carry b (chain0), capture a (chain3)
    dp[0] = (UopDpConfig()
             .enable_alu(AluOp.SUBTRACT, AluInp.PREV_ALU_OUT, AluInp.PREV_DELAY_0)
             .pass_through_delay(0)
             .enable_delay_from_src(DelayInp.PREV_ALU_OUT, 3))
    # s1: alu = b - a; capture (a-b) into chain0
    dp[1] = (UopDpConfig()
             .enable_alu(AluOp.SUBTRACT, AluInp.PREV_DELAY_0, AluInp.PREV_DELAY_3)
             .enable_delay_from_src(DelayInp.PREV_ALU_OUT, 0))
    # s2: alu = max(b-a, a-b)
    dp[2] = UopDpConfig().enable_alu(
        AluOp.MAX, AluInp.PREV_ALU_OUT, AluInp.PREV_DELAY_0)
    for i in range(3, 8):
        dp[i] = UopDpConfig().pass_through_alu()
    return u


def _absdiff_uop_2x():
    """2X_1PORT program: lo on slices 0-2, hi on slices 3-5."""
    from concourse.dve_uop import (
        ENABLE, AluInp, AluOp, DelayInp, InpSel, OutPath, OutSel, Trigger,
        UopConfig, UopDpConfig,
    )

    u = UopConfig()
    u.enable_input(InpSel.SRC_0, 0).enable_input(InpSel.SRC_1, 1)
    u.enable_input(InpSel.SRC_0_HI, 2).enable_input(InpSel.SRC_1_HI, 3)
    u.require_inp0 = ENABLE
    u.require_inp1 = ENABLE
    u.trigger = (Trigger.SRC_TENSOR_DONE, Trigger.NONE, Trigger.NONE)
    u.enable_output(OutSel.DELAY_0, OutPath.WR0_LO)   # lo result rides chain0
    u.enable_output(OutSel.ALU_OUT, OutPath.WR0_HI)   # hi result on ALU lane
    dp = u.datapath_config
    # s0: alu = a_lo - b_lo; carry b_lo(c0), a_hi(c1), b_hi(c2); capture a_lo(c3)
    dp[0] = (UopDpConfig()
             .enable_alu(AluOp.SUBTRACT, AluInp.PREV_ALU_OUT, AluInp.PREV_DELAY_0)
             .pass_through_delay(0, 1, 2)
             .enable_delay_from_src(DelayInp.PREV_ALU_OUT, 3))
    # s1: alu = b_lo - a_lo; capture (a-b)_lo into c0; carry a_hi, b_hi
    dp[1] = (UopDpConfig()
             .enable_alu(AluOp.SUBTRACT, AluInp.PREV_DELAY_0, AluInp.PREV_DELAY_3)
             .enable_delay_from_src(DelayInp.PREV_ALU_OUT, 0)
             .pass_through_delay(1, 2))
    # s2: alu = max -> |a-b|_lo; carry a_hi, b_hi
    dp[2] = (UopDpConfig()
             .enable_alu(AluOp.MAX, AluInp.PREV_ALU_OUT, AluInp.PREV_DELAY_0)
             .pass_through_delay(1, 2))
    # s3: alu = a_hi - b_hi; capture lo result into c0; carry a_hi, b_hi
    dp[3] = (UopDpConfig()
             .enable_alu(AluOp.SUBTRACT, AluInp.PREV_DELAY_1, AluInp.PREV_DELAY_2)
             .enable_delay_from_src(DelayInp.PREV_ALU_OUT, 0)
             .pass_through_delay(1, 2))
    # s4: alu = b_hi - a_hi; carry lo(c0); capture (a-b)_hi into c3
    dp[4] = (UopDpConfig()
             .enable_alu(AluOp.SUBTRACT, AluInp.PREV_DELAY_2, AluInp.PREV_DELAY_1)
             .pass_through_delay(0)
             .enable_delay_from_src(DelayInp.PREV_ALU_OUT, 3))
    # s5: alu = max -> |a-b|_hi; carry lo(c0)
    dp[5] = (UopDpConfig()
             .enable_alu(AluOp.MAX, AluInp.PREV_ALU_OUT, AluInp.PREV_DELAY_3)
             .pass_through_delay(0))
    # s6, s7: pass alu (hi) + chain0 (lo)
    for i in (6, 7):
        dp[i] = UopDpConfig().pass_through_alu().pass_through_delay(0)
    return u


def _get_absdiff_op():
    """Fused |a-b| custom DVE op with a hand-written 2X_1PORT variant."""
    if "absdiff" in _CACHED:
        return _CACHED["absdiff"]
    from concourse import dve_ops
    from concourse.dve_spec import Spec, Src0, Src1, maxx
    from concourse.dve_uop import DveOpSpec

    NAME = "ABSDIFF_ANT"
    for op in dve_ops.OPS:
        if op.name == NAME:
            _CACHED["absdiff"] = op
            return op
    spec = Spec(
        body=maxx(Src0 - Src1, Src1 - Src0),
        reference=lambda in0, in1, s0, s1, imm2: np.abs(
            in0.astype(np.float32) - in1.astype(np.float32)
        ),
    )
    op = dve_ops.DveOp(NAME, spec, subdim=False, uops_sha={})
    dve_ops.OPS.append(op)
    dve_ops.CUSTOM_DVE_SPECS[op.name] = op.spec
    row = dve_ops._CUSTOM_DVE_ROW_BASE + len(dve_ops.OPS) - 1
    dve_ops._SUB_OPCODE_FOR_NAME[op.name] = row
    compiled = DveOpSpec(
        name=NAME,
        opcode=row,
        uops=[_absdiff_uop_1x()],
        uops_2x=[_absdiff_uop_2x()],
        perf_max=1,
        rd1_en=True,
    )
    compiled.validate("v3")
    dve_ops._COMPILE_CACHE[(NAME, "v3")] = compiled
    dve_ops._COMPILE_CACHE[(NAME, "v4")] = compiled
    _CACHED["absdiff"] = op
    return op


# --------------------------------------------------------------------------
# device program
# --------------------------------------------------------------------------

def make_pools(tc, ctx, rep=0):
    sfx = f"_{rep}"
    singles = ctx.enter_context(tc.tile_pool(name="singles" + sfx, bufs=1))
    ps = ctx.enter_context(tc.tile_pool(name="ps" + sfx, bufs=3, space="PSUM"))
    adp = ctx.enter_context(tc.tile_pool(name="adp" + sfx, bufs=4))
    tp = ctx.enter_context(tc.tile_pool(name="tp" + sfx, bufs=3))
    return singles, ps, adp, tp


def build_body(tc, outs, ins, rep=0, pools=None):
    """Trace the per-core Tile program.

    ins (all f16, flat [128, cols] so each is one DMA):
          xT   [128,2048]  xT[f, fh*1024+n*32+hwl] = x[n, 32k+hwl, fh*128+f]
          tw   [128,1024]  tw[f, (fh*4+q)*128+b*8+c] = T[fh*128+f,16q+b,c]
          ones [128,1024]  ones[b*8+c, s*128+col] = (col == 16s+b)
          onescol [128,256] onescol[16s+b, q*64+16q'+b'] = (q'==q and b'==b)
    outs: orow [128,512] f16  orow[16s+b, (q*4+g)*32+hw] = tail-part of
                              o_b[8g+s, hw, 16q+b]: sum over j>=8g+8 for g<3,
                              over j>=24 (whole band) for g=3
          ocol [64,1024] f16  ocol[16q+b, h*256+jj*32+hw] =
                              sum_{i<min(8h+8,24)} exp(-norm[i, 8h+jj]), h 0..3
    """
    from contextlib import ExitStack

    import concourse.bass as bass
    import concourse.mybir as mybir

    nc = tc.nc
    f16 = mybir.dt.float16
    f32 = mybir.dt.float32

    xT_d, tw_d, ones_d, onescol_d = ins["xT"], ins["tw"], ins["ones"], ins["onescol"]
    orow_d, ocol_d = outs["orow"], outs["ocol"]

    with ExitStack() as ctx:
        if pools is None:
            pools = make_pools(tc, ctx, rep)
        singles, ps, adp, tp = pools

        def gp_add(out, in0, in1):
            nc.gpsimd.tensor_tensor(
                out=out, in0=in0, in1=in1, op=mybir.AluOpType.add)

        def tail_tree(E, jstart, jcount, out_ap):
            """Sum E cols [jstart*HWL, (jstart+jcount)*HWL) over j into out_ap.

            f32 intermediates: strict-triangle fakes push partial sums past
            8.0 where the fp16 quantum (2^-7) would dominate the error."""
            sc = tp.tile([128, 1024], f32, tag="tt")
            cur_t, cur_o, n, sc_off = E, jstart * HWL, jcount, 0
            while n > 3:
                h = n // 2
                dest = sc[:, sc_off:sc_off + h * HWL]
                gp_add(dest, cur_t[:, cur_o:cur_o + h * HWL],
                       cur_t[:, cur_o + h * HWL:cur_o + 2 * h * HWL])
                cur_t, cur_o, n = sc, sc_off, h
                sc_off += h * HWL
            if n == 3:
                t = sc[:, sc_off:sc_off + HWL]
                gp_add(t, cur_t[:, cur_o:cur_o + HWL],
                       cur_t[:, cur_o + HWL:cur_o + 2 * HWL])
                gp_add(out_ap, t, cur_t[:, cur_o + 2 * HWL:cur_o + 3 * HWL])
            else:
                assert n == 2
                gp_add(out_ap, cur_t[:, cur_o:cur_o + HWL],
                       cur_t[:, cur_o + HWL:cur_o + 2 * HWL])

        # ---- PE warm-up: ramp the tensor engine to full pstate while the
        # DMAs land (reads a DVE-zeroed tile, result discarded) -----------
        z = singles.tile([128, 512], f16, tag="z")
        (nc.gpsimd if TAIL_ENGINE == "pool" else nc.vector).memset(z, 0.0)
        wps = ps.tile([128, 512], f32, tag="pcol", bufs=2)
        for _ in range(7):
            nc.tensor.matmul(wps[:], lhsT=z[:, 0:128], rhs=z[:],
                             start=True, stop=True)

        # ---- loads (5 DMAs, spread over 4 DGE queues so generation
        # overlaps; xT split in halves so stage B starts early) -----------
        twall = singles.tile([128, 8 * 128], f16, tag="twall")
        nc.sync.dma_start(out=twall, in_=tw_d)
        xTall = singles.tile([128, FH * N * HWL], f16, tag="xTall")
        xTv = xTall.rearrange("p (fh c) -> p fh c", fh=FH)
        xTd = xT_d.rearrange("p (fh c) -> p fh c", fh=FH)
        nc.scalar.dma_start(out=xTv[:, :, 512:1024], in_=xTd[:, :, 512:1024])
        nc.sync.dma_start(out=xTv[:, :, 0:512], in_=xTd[:, :, 0:512])
        dge2 = nc.gpsimd if TAIL_ENGINE == "pool" else nc.scalar
        onesall = singles.tile([128, 8 * 128], f16, tag="onesall")
        dge2.dma_start(out=onesall, in_=ones_d)
        onescolall = singles.tile([128, Q * 64], f16, tag="onescolall")
        dge2.dma_start(out=onescolall, in_=onescol_d)
        ones_s = [onesall[:, s * 128:(s + 1) * 128] for s in range(8)]
        onescol_s = [onescolall[:, q * 64:(q + 1) * 64] for q in range(Q)]

        # ---- stage B: M2[q] = einsum, [(b,c) part, (n,hw) free] ----------
        M2 = []
        for q in range(Q):
            pm = ps.tile([128, N * HWL], f32, tag="nrm")
            m2 = singles.tile([128, N * HWL], f16, tag=f"m2{q}")
            for sub in (1, 0):   # sub1 first: the first band (g2) reads only it
                sl = slice(sub * 512, (sub + 1) * 512)
                for fh in range(FH):
                    nc.tensor.matmul(
                        pm[:, sl],
                        lhsT=twall[:, (fh * 4 + q) * 128:(fh * 4 + q + 1) * 128],
                        rhs=xTall[:, fh * 1024 + sub * 512:
                                  fh * 1024 + (sub + 1) * 512],
                        start=(fh == 0), stop=(fh == 1),
                    )
                nc.scalar.copy(out=m2[:, sl], in_=pm[:, sl])
            M2.append(m2)

        # ---- stage C: bands ---------------------------------------------
        orow_sb = singles.tile([128, Q * G * HWL], f32, tag="orow")
        ocol_sb = singles.tile([64, G * GS * HWL], f32, tag="ocol")
        E_tiles = {}        # (q, g) -> E tile [(s,b) part, (j,hw) free] f16
        done_bands = set()

        def col_part(h, gmax):
            """sum_{i<8(gmax+1)} E[i, j in group h] via PE stripes."""
            pcol = ps.tile([64, GS * HWL], f32, tag="pcol", bufs=2)
            nmm = Q * (gmax + 1)
            k = 0
            for q in range(Q):
                for gp in range(gmax + 1):
                    Ejs = E_tiles[(q, gp)]
                    c0 = (GS * h - GS * gp) * HWL
                    nc.tensor.matmul(
                        pcol[:], lhsT=onescol_s[q],
                        rhs=Ejs[:, c0:c0 + GS * HWL],
                        start=(k == 0), stop=(k == nmm - 1),
                    )
                    k += 1
            nc.scalar.copy(
                out=ocol_sb[:, h * GS * HWL:(h + 1) * GS * HWL],
                in_=pcol[:],
            )

        # band order: g2 first (its absdiff reads only M2 cols [512:1024],
        # available one copy earlier); g3 last so the final column part has
        # the shortest dependent tail.
        for g in (2, 1, 0, 3):
            j0 = GS * g
            L = N - j0                       # j extent of this band
            for q in range(Q):
                m2v = M2[q].rearrange("p (n hw) -> p n hw", n=N)
                # |M2[:, j, hw] - M2[:, i, hw]| for i in band, j in [j0, N)
                ad = adp.tile([128, GS * L * HWL], f16, tag="ad")
                # (j, hw) of M2 is contiguous -> flat src0; src1 broadcasts
                # the i-row over j ([0, L] dim). One op per i (custom DVE
                # ops allow at most 2 free dims).
                for s in range(GS):
                    # strict triangle: row i = 8g+s only computes j > i
                    # (j >= i for s=0, whose stripe must span the full PSUM
                    # extent so its start=True zeroes everything). Skipped
                    # cells read 0 from PSUM, so E holds exp(0)=1 there —
                    # which IS the correct diagonal value, and a constant
                    # the host subtracts everywhere else.
                    js = s if s == 0 else s + 1   # band-relative j start
                    if js >= L:
                        continue                  # row 31: nothing above diag
                    i = j0 + s
                    sj = m2v[:, j0 + js:N, :]
                    src0 = bass.AP(
                        tensor=sj.tensor, offset=sj.offset,
                        ap=[list(sj.ap[0]), [1, (L - js) * HWL]],
                    )
                    si = m2v[:, i, :]
                    src1 = bass.AP(
                        tensor=si.tensor, offset=si.offset,
                        ap=[list(si.ap[0]), [0, L - js], list(si.ap[1])],
                    )
                    bi = nc.vector._custom_dve(
                        _get_absdiff_op(),
                        out=ad[:, (s * L + js) * HWL:(s + 1) * L * HWL],
                        in0=src0, in1=src1,
                    )
                    bi.ins.perf_max = PERF_MAX

                # c-reduce: 8 i-stripes -> psum [(s,b) part, (j,hw) free].
                # Stripe s only feeds cols it computed (j > i; j >= i for
                # s=0, whose start=True must zero the full extent). Row 31
                # (g3 s=7) has no data and emits nothing.
                nrm = ps.tile([128, L * HWL], f32, tag="nrm")
                emitted = [s for s in range(8)
                           if (s if s == 0 else s + 1) < L]
                for s in emitted:
                    c0a = (s if s == 0 else s + 1) * HWL
                    nchunks = [(c0a, min(512, L * HWL) - c0a)]
                    if L * HWL > 512:
                        nchunks.append((512, L * HWL - 512))
                    for c0, clen in nchunks:
                        nc.tensor.matmul(
                            nrm[:, c0:c0 + clen], lhsT=ones_s[s],
                            rhs=ad[:, s * L * HWL + c0:s * L * HWL + c0 + clen],
                            start=(s == 0), stop=(s == emitted[-1]),
                        )

                E = singles.tile([128, L * HWL], f16, tag=f"E{q}{g}")
                nc.scalar.activation(
                    out=E, in_=nrm[:],
                    func=mybir.ActivationFunctionType.Exp, scale=-1.0,
                )
                E_tiles[(q, g)] = E

                # row part: sum the WHOLE band (j >= 8g) — with strict
                # stripes this is s fakes + diagonal + all j>i terms; the
                # column part supplies i<n. Fakes/diagonal-dup are constants
                # the host subtracts. g<3 on Pool add-trees (keeps the DVE
                # on absdiff); g=3 as a DVE reduce — the DVE is idle by then
                # and the Pool trees would serialize into the tail.
                oslice = orow_sb[:, (q * G + g) * HWL:(q * G + g + 1) * HWL]
                if g < 3 and TAIL_ENGINE == "pool":
                    tail_tree(E, 0, L, oslice)
                else:
                    Ev = E.rearrange("p (j hw) -> p hw j", j=L)
                    nc.vector.tensor_reduce(
                        out=oslice, in_=Ev, axis=mybir.AxisListType.X,
                        op=mybir.AluOpType.add,
                    )

            # emit any column parts whose band prerequisites are now done;
            # h<3 slices of ocol ship mid-stream, h=3 ships in the tail
            done_bands.add(g)
            for h in range(G):
                if ("col", h) not in done_bands and all(
                        gp in done_bands for gp in range(h + 1)):
                    col_part(h, h)
                    done_bands.add(("col", h))
            if all(("col", h) in done_bands for h in range(3)) \
                    and "ocol_dma" not in done_bands:
                nc.scalar.dma_start(out=ocol_d[:, 0:3 * GS * HWL],
                                    in_=ocol_sb[:, 0:3 * GS * HWL])
                done_bands.add("ocol_dma")

        # ship the g<3 row parts as soon as their trees finish; only the
        # tiny g3 slice (computed in the tail) goes in the final DMA
        orv_d = orow_d.rearrange("p (q g hw) -> p q g hw", q=Q, g=G)
        orv_s = orow_sb.rearrange("p (q g hw) -> p q g hw", q=Q, g=G)
        nc.sync.dma_start(out=orv_d[:, :, 0:3], in_=orv_s[:, :, 0:3])
        nc.sync.dma_start(out=orv_d[:, :, 3], in_=orv_s[:, :, 3])
        nc.scalar.dma_start(out=ocol_d[:, 3 * GS * HWL:],
                            in_=ocol_sb[:, 3 * GS * HWL:])


# --------------------------------------------------------------------------
# host side
# --------------------------------------------------------------------------

def prep_inputs(x, T):
    """Shared (core-independent) device inputs; xTg is sliced per core."""
    # xTg[f, fh, n, hw] = x[n, hw, fh*128+f]
    xTg = np.ascontiguousarray(
        x.reshape(N, HW, FH, 128).transpose(3, 2, 0, 1)
    ).astype(np.float16)
    # tw[f, (fh,q), b*8+c]
    tw = T.reshape(FH, 128, Q, 16, C).transpose(1, 0, 2, 3, 4)
    tw_in = np.ascontiguousarray(tw.reshape(128, FH * Q * 128)).astype(np.float16)
    ones_in = np.zeros((128, 8, 128), np.float16)
    for s in range(8):
        for b in range(16):
            ones_in[b * 8:(b + 1) * 8, s, 16 * s + b] = 1.0
    ones_in = ones_in.reshape(128, 1024)
    onescol_in = np.zeros((128, Q, 64), np.float16)
    for q in range(Q):
        for s in range(8):
            for b in range(16):
                onescol_in[16 * s + b, q, 16 * q + b] = 1.0
    onescol_in = onescol_in.reshape(128, Q * 64)
    return xTg, tw_in, ones_in, onescol_in


def core_in_map(xTg, tw_in, ones_in, onescol_in, k):
    xT = np.ascontiguousarray(
        xTg[:, :, :, k * HWL:(k + 1) * HWL]
    ).reshape(128, FH * N * HWL)
    return {"xT": xT, "tw": tw_in, "ones": ones_in, "onescol": onescol_in}


def gather_ob(core_outs):
    """core_outs: list of 8 dicts {orow:[128,512], ocol:[64,1024]} -> o_b."""
    # strict-triangle fakes: exp(0)=1 cells the device summed, removed here.
    # ocol self-stripes add (7-jj) fakes per column jj. orow full-band sums
    # add s fakes, and the diagonal is counted in both parts (+1).
    colfix = ((7 - np.arange(GS)).astype(np.float32))[None, :, None, None]
    rowfix = (np.arange(8) + 1.0)[None, :, None, None, None].astype(np.float32)
    obs = []
    for res in core_outs:
        c = res["ocol"].astype(np.float32).reshape(Q, 16, G, GS, HWL)
        # [q, b2, h, jj, hw] -> [n=(h,jj), hw, b=(q,b2)]
        ob = np.ascontiguousarray(
            c.transpose(2, 3, 4, 0, 1)).reshape(G, GS, HWL, B)
        ob -= colfix
        ob = ob.reshape(N, HWL, B)
        r = res["orow"].astype(np.float32).reshape(8, 16, Q, G, HWL)
        # [s, b2, q, g, hw] -> [n=(g,s), hw, b=(q,b2)]
        rr = r.transpose(3, 0, 4, 2, 1).copy()      # [g, s, hw, q, b2]
        rr -= rowfix
        ob += rr.reshape(N, HWL, B)
        obs.append(ob)
    # core k owns hw slice k -> stack on hw axis
    full = np.stack(obs, axis=1).reshape(N, HW, B)
    return full.reshape(N, 16, 16, B)


_CACHED = {}


def _get_program(reps=1, loop=None):
    key = ("nc", reps, loop)
    if key in _CACHED:
        return _CACHED[key]
    from contextlib import ExitStack
    import concourse.bacc as bacc
    import concourse.mybir as mybir
    import concourse.tile as tile

    nc = bacc.Bacc("TRN2", target_bir_lowering=False, debug=False,
                   num_devices=CORES)
    f16, f32 = mybir.dt.float16, mybir.dt.float32
    ins = {
        "xT": nc.dram_tensor("xT", [128, FH * N * HWL], f16,
                             kind="ExternalInput").ap(),
        "tw": nc.dram_tensor("tw", [128, FH * Q * 128], f16,
                             kind="ExternalInput").ap(),
        "ones": nc.dram_tensor("ones", [128, 1024], f16,
                               kind="ExternalInput").ap(),
        "onescol": nc.dram_tensor("onescol", [128, Q * 64], f16,
                                  kind="ExternalInput").ap(),
    }
    outs = {
        "orow": nc.dram_tensor("orow", [128, Q * G * HWL], f32,
                               kind="ExternalOutput").ap(),
        "ocol": nc.dram_tensor("ocol", [64, G * GS * HWL], f32,
                               kind="ExternalOutput").ap(),
    }
    with tile.TileContext(nc) as tc:
        if loop:
            with ExitStack() as ctx:
                pools = make_pools(tc, ctx)
                with tc.For_i(0, loop, 1,
                              hint_engines=(mybir.EngineType.PE,
                                            mybir.EngineType.DVE)):
                    build_body(tc, outs, ins, pools=pools)
        else:
            for r in range(reps):
                build_body(tc, outs, ins, rep=r)
    nc.compile()
    _CACHED[key] = nc
    return nc


def kernel(x, T):
    x = np.asarray(x, dtype=np.float32)
    T = np.asarray(T, dtype=np.float32)
    from concourse.bass_utils import run_bass_kernel_spmd

    nc = _get_program()
    xTg, tw_in, ones_in, onescol_in = prep_inputs(x, T)
    in_maps = [core_in_map(xTg, tw_in, ones_in, onescol_in, k)
               for k in range(CORES)]
    res = run_bass_kernel_spmd(nc, in_maps, core_ids=list(range(CORES)))
    ob = gather_ob(res.results)
    return np.concatenate([x, ob], axis=3)


# revision 69
# speedup vs baseline: 1.0016x; 1.0016x over previous
"""MiniBatchDiscrimination Trainium2 kernel (hw-sharded, symmetric bands).

reference:
    M = einsum('nhwf,fbc->nhwbc', x, T)          # [N,H,W,B,C]
    norm = sum_c |M[i] - M[j]|                   # [N,N,H,W,B]
    o_b  = sum_j exp(-norm)                      # [N,H,W,B]
    out  = concat([x, o_b], axis=3)              # [N,H,W,F+B]

Sharding: the pairwise block is over n only, so hw (=h*16+w, 256 values) is
embarrassingly parallel: core k owns hw in [32k, 32k+32) and computes the
FULL 32x32 pairwise block for its slice. No redundant M compute, 8x less
input DMA than data-parallel-over-n.

Symmetry: norm[i,j] == norm[j,i]. Rows are processed in 4 bands g (i in
[8g,8g+8)); row i computes STRICTLY j >= i (0.516x of the full block — the
symmetric minimum incl. diagonal). Skipped sub-diagonal cells of a band
read 0 from PSUM (the s=0 stripe's start=True zeroes them), so E holds
exp(0)=1 there — constants the host subtracts. Then
  o_b[n], n in group h = column sums over i<=n of E[q,g'] for g' <= h
    (PE stripe-matmuls with per-q ones matrices; + (7-jj) fakes)
  + full-band row sum over j of E[q,h] at row n (Pool add-trees, g=3 as a
    DVE reduce in the tail; + s fakes + a double-counted diagonal).
The two parts live in different layouts and are added on the host, which
also subtracts the constant fake terms.

Per-core device layout: M2[q] [(b16,c8) part, (n32,hw32) free] fp16 so that
  - M-compute is a plain matmul (lhsT = T-tile [f,(b,c)], rhs = xT [f,(n,hw)])
  - |M_j - M_i| is one custom DVE op (2X_1PORT, 2 elem/cycle) per (q, band,
    row) over the contiguous (j,hw) range, broadcasting the i-row via a
    0-stride AP dim
  - the c-reduction contracts the partition axis on the TensorEngine with
    stripe-ones matrices, 8 i-stripes accumulating into one PSUM tile
    [(i8,b16) part, (j,hw) free]
  - exp(-norm) is one ACT pass per (q, band).
Engines in steady state: DVE absdiff (the critical path), PE stripes +
column parts, ACT M2-copies + exp, Pool row-sum trees + 2 input DMAs.
A few zero matmuls at t=0 ramp the PE out of its low-clock pstate while
the (4-queue-parallel) input DMAs land.
"""

import os
import sys

for _p in ("/opt/trn_rl_repo", "/opt/pypackages"):
    if _p not in sys.path and os.path.isdir(_p):
        sys.path.append(_p)

import numpy as np

N, F, B, C = 32, 256, 64, 8
HW = 256        # total hw positions
HWL = 32        # hw positions per core
CORES = 8
FH = 2          # f in two partition halves of 128
Q = 4           # b-quarters of 16
G = 4           # i-bands of 8
GS = 8          # band size
F16 = "float16"

PERF_MAX = 1
# "pool": row sums as gpsimd add-trees; "dve": as DVE tensor_reduce.
# (A/B knob: CoreSim models Q7 launch at ~95ns; if real-hw gpsimd ops are
# slower, "dve" trades ~10us of modeled DVE time for no Pool ops at all.)
TAIL_ENGINE = "pool"


def _absdiff_uop_1x():
    """REGULAR program: |a-b| via SUB, reverse-SUB, MAX on slices 0-2."""
    from concourse.dve_uop import (
        ENABLE, AluInp, AluOp, DelayInp, InpSel, OutPath, OutSel, Trigger,
        UopConfig, UopDpConfig,
    )

    u = UopConfig()
    u.enable_input(InpSel.SRC_0, 0).enable_input(InpSel.SRC_1, 1)
    u.require_inp0 = ENABLE
    u.require_inp1 = ENABLE
    u.trigger = (Trigger.SRC_TENSOR_DONE, Trigger.NONE, Trigger.NONE)
    u.enable_output(OutSel.ALU_OUT, OutPath.WR0_LO)
    dp = u.datapath_config
    # s0: alu = a - b; carry b (chain0), capture a (chain3)
    dp[0] = (UopDpConfig()
             .enable_alu(AluOp.SUBTRACT, AluInp.PREV_ALU_OUT, AluInp.PREV_DELAY_0)
             .pass_through_delay(0)
             .enable_delay_from_src(DelayInp.PREV_ALU_OUT, 3))
    # s1: alu = b - a; capture (a-b) into chain0
    dp[1] = (UopDpConfig()
             .enable_alu(AluOp.SUBTRACT, AluInp.PREV_DELAY_0, AluInp.PREV_DELAY_3)
             .enable_delay_from_src(DelayInp.PREV_ALU_OUT, 0))
    # s2: alu = max(b-a, a-b)
    dp[2] = UopDpConfig().enable_alu(
        AluOp.MAX, AluInp.PREV_ALU_OUT, AluInp.PREV_DELAY_0)
    for i in range(3, 8):
        dp[i] = UopDpConfig().pass_through_alu()
    return u


def _absdiff_uop_2x():
    """2X_1PORT program: lo on slices 0-2, hi on slices 3-5."""
    from concourse.dve_uop import (
        ENABLE, AluInp, AluOp, DelayInp, InpSel, OutPath, OutSel, Trigger,
        UopConfig, UopDpConfig,
    )

    u = UopConfig()
    u.enable_input(InpSel.SRC_0, 0).enable_input(InpSel.SRC_1, 1)
    u.enable_input(InpSel.SRC_0_HI, 2).enable_input(InpSel.SRC_1_HI, 3)
    u.require_inp0 = ENABLE
    u.require_inp1 = ENABLE
    u.trigger = (Trigger.SRC_TENSOR_DONE, Trigger.NONE, Trigger.NONE)
    u.enable_output(OutSel.DELAY_0, OutPath.WR0_LO)   # lo result rides chain0
    u.enable_output(OutSel.ALU_OUT, OutPath.WR0_HI)   # hi result on ALU lane
    dp = u.datapath_config
    # s0: alu = a_lo - b_lo; carry b_lo(c0), a_hi(c1), b_hi(c2); capture a_lo(c3)
    dp[0] = (UopDpConfig()
             .enable_alu(AluOp.SUBTRACT, AluInp.PREV_ALU_OUT, AluInp.PREV_DELAY_0)
             .pass_through_delay(0, 1, 2)
             .enable_delay_from_src(DelayInp.PREV_ALU_OUT, 3))
    # s1: alu = b_lo - a_lo; capture (a-b)_lo into c0; carry a_hi, b_hi
    dp[1] = (UopDpConfig()
             .enable_alu(AluOp.SUBTRACT, AluInp.PREV_DELAY_0, AluInp.PREV_DELAY_3)
             .enable_delay_from_src(DelayInp.PREV_ALU_OUT, 0)
             .pass_through_delay(1, 2))
    # s2: alu = max -> |a-b|_lo; carry a_hi, b_hi
    dp[2] = (UopDpConfig()
             .enable_alu(AluOp.MAX, AluInp.PREV_ALU_OUT, AluInp.PREV_DELAY_0)
             .pass_through_delay(1, 2))
    # s3: alu = a_hi - b_hi; capture lo result into c0; carry a_hi, b_hi
    dp[3] = (UopDpConfig()
             .enable_alu(AluOp.SUBTRACT, AluInp.PREV_DELAY_1, AluInp.PREV_DELAY_2)
             .enable_delay_from_src(DelayInp.PREV_ALU_OUT, 0)
             .pass_through_delay(1, 2))
    # s4: alu = b_hi - a_hi; carry lo(c0); capture (a-b)_hi into c3
    dp[4] = (UopDpConfig()
             .enable_alu(AluOp.SUBTRACT, AluInp.PREV_DELAY_2, AluInp.PREV_DELAY_1)
             .pass_through_delay(0)
             .enable_delay_from_src(DelayInp.PREV_ALU_OUT, 3))
    # s5: alu = max -> |a-b|_hi; carry lo(c0)
    dp[5] = (UopDpConfig()
             .enable_alu(AluOp.MAX, AluInp.PREV_ALU_OUT, AluInp.PREV_DELAY_3)
             .pass_through_delay(0))
    # s6, s7: pass alu (hi) + chain0 (lo)
    for i in (6, 7):
        dp[i] = UopDpConfig().pass_through_alu().pass_through_delay(0)
    return u


def _get_absdiff_op():
    """Fused |a-b| custom DVE op with a hand-written 2X_1PORT variant."""
    if "absdiff" in _CACHED:
        return _CACHED["absdiff"]
    from concourse import dve_ops
    from concourse.dve_spec import Spec, Src0, Src1, maxx
    from concourse.dve_uop import DveOpSpec

    NAME = "ABSDIFF_ANT"
    for op in dve_ops.OPS:
        if op.name == NAME:
            _CACHED["absdiff"] = op
            return op
    spec = Spec(
        body=maxx(Src0 - Src1, Src1 - Src0),
        reference=lambda in0, in1, s0, s1, imm2: np.abs(
            in0.astype(np.float32) - in1.astype(np.float32)
        ),
    )
    op = dve_ops.DveOp(NAME, spec, subdim=False, uops_sha={})
    dve_ops.OPS.append(op)
    dve_ops.CUSTOM_DVE_SPECS[op.name] = op.spec
    row = dve_ops._CUSTOM_DVE_ROW_BASE + len(dve_ops.OPS) - 1
    dve_ops._SUB_OPCODE_FOR_NAME[op.name] = row
    compiled = DveOpSpec(
        name=NAME,
        opcode=row,
        uops=[_absdiff_uop_1x()],
        uops_2x=[_absdiff_uop_2x()],
        perf_max=1,
        rd1_en=True,
    )
    compiled.validate("v3")
    dve_ops._COMPILE_CACHE[(NAME, "v3")] = compiled
    dve_ops._COMPILE_CACHE[(NAME, "v4")] = compiled
    _CACHED["absdiff"] = op
    return op


# --------------------------------------------------------------------------
# device program
# --------------------------------------------------------------------------

def make_pools(tc, ctx, rep=0):
    sfx = f"_{rep}"
    singles = ctx.enter_context(tc.tile_pool(name="singles" + sfx, bufs=1))
    ps = ctx.enter_context(tc.tile_pool(name="ps" + sfx, bufs=3, space="PSUM"))
    adp = ctx.enter_context(tc.tile_pool(name="adp" + sfx, bufs=4))
    tp = ctx.enter_context(tc.tile_pool(name="tp" + sfx, bufs=3))
    return singles, ps, adp, tp


def build_body(tc, outs, ins, rep=0, pools=None):
    """Trace the per-core Tile program.

    ins (all f16, flat [128, cols] so each is one DMA):
          xT   [128,2048]  xT[f, fh*1024+n*32+hwl] = x[n, 32k+hwl, fh*128+f]
          tw   [128,1024]  tw[f, (fh*4+q)*128+b*8+c] = T[fh*128+f,16q+b,c]
          ones [128,1024]  ones[b*8+c, s*128+col] = (col == 16s+b)
          onescol [128,256] onescol[16s+b, q*64+16q'+b'] = (q'==q and b'==b)
    outs: orow [128,512] f16  orow[16s+b, (q*4+g)*32+hw] = tail-part of
                              o_b[8g+s, hw, 16q+b]: sum over j>=8g+8 for g<3,
                              over j>=24 (whole band) for g=3
          ocol [64,1024] f16  ocol[16q+b, h*256+jj*32+hw] =
                              sum_{i<min(8h+8,24)} exp(-norm[i, 8h+jj]), h 0..3
    """
    from contextlib import ExitStack

    import concourse.bass as bass
    import concourse.mybir as mybir

    nc = tc.nc
    f16 = mybir.dt.float16
    f32 = mybir.dt.float32

    xT_d, tw_d, ones_d, onescol_d = ins["xT"], ins["tw"], ins["ones"], ins["onescol"]
    orow_d, ocol_d = outs["orow"], outs["ocol"]

    with ExitStack() as ctx:
        if pools is None:
            pools = make_pools(tc, ctx, rep)
        singles, ps, adp, tp = pools

        def gp_add(out, in0, in1):
            nc.gpsimd.tensor_tensor(
                out=out, in0=in0, in1=in1, op=mybir.AluOpType.add)

        def tail_tree(E, jstart, jcount, out_ap):
            """Sum E cols [jstart*HWL, (jstart+jcount)*HWL) over j into out_ap.

            f32 intermediates: strict-triangle fakes push partial sums past
            8.0 where the fp16 quantum (2^-7) would dominate the error."""
            sc = tp.tile([128, 1024], f32, tag="tt")
            cur_t, cur_o, n, sc_off = E, jstart * HWL, jcount, 0
            while n > 3:
                h = n // 2
                dest = sc[:, sc_off:sc_off + h * HWL]
                gp_add(dest, cur_t[:, cur_o:cur_o + h * HWL],
                       cur_t[:, cur_o + h * HWL:cur_o + 2 * h * HWL])
                cur_t, cur_o, n = sc, sc_off, h
                sc_off += h * HWL
            if n == 3:
                t = sc[:, sc_off:sc_off + HWL]
                gp_add(t, cur_t[:, cur_o:cur_o + HWL],
                       cur_t[:, cur_o + HWL:cur_o + 2 * HWL])
                gp_add(out_ap, t, cur_t[:, cur_o + 2 * HWL:cur_o + 3 * HWL])
            else:
                assert n == 2
                gp_add(out_ap, cur_t[:, cur_o:cur_o + HWL],
                       cur_t[:, cur_o + HWL:cur_o + 2 * HWL])

        # ---- PE warm-up: ramp the tensor engine to full pstate while the
        # DMAs land (reads a DVE-zeroed tile, result discarded) -----------
        z = singles.tile([128, 512], f16, tag="z")
        (nc.gpsimd if TAIL_ENGINE == "pool" else nc.vector).memset(z, 0.0)
        wps = ps.tile([128, 512], f32, tag="pcol", bufs=2)
        for _ in range(7):
            nc.tensor.matmul(wps[:], lhsT=z[:, 0:128], rhs=z[:],
                             start=True, stop=True)

        # ---- loads (5 DMAs, spread over 4 DGE queues so generation
        # overlaps; xT split in halves so stage B starts early) -----------
        twall = singles.tile([128, 8 * 128], f16, tag="twall")
        nc.sync.dma_start(out=twall, in_=tw_d)
        xTall = singles.tile([128, FH * N * HWL], f16, tag="xTall")
        xTv = xTall.rearrange("p (fh c) -> p fh c", fh=FH)
        xTd = xT_d.rearrange("p (fh c) -> p fh c", fh=FH)
        nc.scalar.dma_start(out=xTv[:, :, 512:1024], in_=xTd[:, :, 512:1024])
        nc.sync.dma_start(out=xTv[:, :, 0:512], in_=xTd[:, :, 0:512])
        dge2 = nc.gpsimd if TAIL_ENGINE == "pool" else nc.scalar
        onesall = singles.tile([128, 8 * 128], f16, tag="onesall")
        dge2.dma_start(out=onesall, in_=ones_d)
        onescolall = singles.tile([128, Q * 64], f16, tag="onescolall")
        dge2.dma_start(out=onescolall, in_=onescol_d)
        ones_s = [onesall[:, s * 128:(s + 1) * 128] for s in range(8)]
        onescol_s = [onescolall[:, q * 64:(q + 1) * 64] for q in range(Q)]

        # ---- stage B: M2[q] = einsum, [(b,c) part, (n,hw) free] ----------
        M2 = []
        for q in range(Q):
            pm = ps.tile([128, N * HWL], f32, tag="nrm")
            m2 = singles.tile([128, N * HWL], f16, tag=f"m2{q}")
            for sub in (1, 0):   # sub1 first: the first band (g2) reads only it
                sl = slice(sub * 512, (sub + 1) * 512)
                for fh in range(FH):
                    nc.tensor.matmul(
                        pm[:, sl],
                        lhsT=twall[:, (fh * 4 + q) * 128:(fh * 4 + q + 1) * 128],
                        rhs=xTall[:, fh * 1024 + sub * 512:
                                  fh * 1024 + (sub + 1) * 512],
                        start=(fh == 0), stop=(fh == 1),
                    )
                nc.scalar.copy(out=m2[:, sl], in_=pm[:, sl])
            M2.append(m2)

        # ---- stage C: bands ---------------------------------------------
        orow_sb = singles.tile([128, Q * G * HWL], f32, tag="orow")
        ocol_sb = singles.tile([64, G * GS * HWL], f32, tag="ocol")
        E_tiles = {}        # (q, g) -> E tile [(s,b) part, (j,hw) free] f16
        done_bands = set()

        def col_part(h, gmax):
            """sum_{i<8(gmax+1)} E[i, j in group h] via PE stripes."""
            pcol = ps.tile([64, GS * HWL], f32, tag="pcol", bufs=2)
            nmm = Q * (gmax + 1)
            k = 0
            for q in range(Q):
                for gp in range(gmax + 1):
                    Ejs = E_tiles[(q, gp)]
                    c0 = (GS * h - GS * gp) * HWL
                    nc.tensor.matmul(
                        pcol[:], lhsT=onescol_s[q],
                        rhs=Ejs[:, c0:c0 + GS * HWL],
                        start=(k == 0), stop=(k == nmm - 1),
                    )
                    k += 1
            nc.scalar.copy(
                out=ocol_sb[:, h * GS * HWL:(h + 1) * GS * HWL],
                in_=pcol[:],
            )

        # band order: g2 first (its absdiff reads only M2 cols [512:1024],
        # available one copy earlier); g3 last so the final column part has
        # the shortest dependent tail.
        for g in (2, 1, 0, 3):
            j0 = GS * g
            L = N - j0                       # j extent of this band
            for q in range(Q):
                m2v = M2[q].rearrange("p (n hw) -> p n hw", n=N)
                # |M2[:, j, hw] - M2[:, i, hw]| for i in band, j in [j0, N)
                ad = adp.tile([128, GS * L * HWL], f16, tag="ad")
                # (j, hw) of M2 is contiguous -> flat src0; src1 broadcasts
                # the i-row over j ([0, L] dim). One op per i (custom DVE
                # ops allow at most 2 free dims).
                for s in range(GS):
                    # strict triangle: row i = 8g+s only computes j > i
                    # (j >= i for s=0, whose stripe must span the full PSUM
                    # extent so its start=True zeroes everything). Skipped
                    # cells read 0 from PSUM, so E holds exp(0)=1 there —
                    # which IS the correct diagonal value, and a constant
                    # the host subtracts everywhere else.
                    js = s if s == 0 else s + 1   # band-relative j start
                    if js >= L:
                        continue                  # row 31: nothing above diag
                    i = j0 + s
                    sj = m2v[:, j0 + js:N, :]
                    src0 = bass.AP(
                        tensor=sj.tensor, offset=sj.offset,
                        ap=[list(sj.ap[0]), [1, (L - js) * HWL]],
                    )
                    si = m2v[:, i, :]
                    src1 = bass.AP(
                        tensor=si.tensor, offset=si.offset,
                        ap=[list(si.ap[0]), [0, L - js], list(si.ap[1])],
                    )
                    bi = nc.vector._custom_dve(
                        _get_absdiff_op(),
                        out=ad[:, (s * L + js) * HWL:(s + 1) * L * HWL],
                        in0=src0, in1=src1,
                    )
                    bi.ins.perf_max = PERF_MAX

                # c-reduce: 8 i-stripes -> psum [(s,b) part, (j,hw) free].
                # Stripe s only feeds cols it computed (j > i; j >= i for
                # s=0, whose start=True must zero the full extent). Row 31
                # (g3 s=7) has no data and emits nothing.
                nrm = ps.tile([128, L * HWL], f32, tag="nrm")
                emitted = [s for s in range(8)
                           if (s if s == 0 else s + 1) < L]
                for s in emitted:
                    c0a = (s if s == 0 else s + 1) * HWL
                    nchunks = [(c0a, min(512, L * HWL) - c0a)]
                    if L * HWL > 512:
                        nchunks.append((512, L * HWL - 512))
                    for c0, clen in nchunks:
                        nc.tensor.matmul(
                            nrm[:, c0:c0 + clen], lhsT=ones_s[s],
                            rhs=ad[:, s * L * HWL + c0:s * L * HWL + c0 + clen],
                            start=(s == 0), stop=(s == emitted[-1]),
                        )

                E = singles.tile([128, L * HWL], f16, tag=f"E{q}{g}")
                nc.scalar.activation(
                    out=E, in_=nrm[:],
                    func=mybir.ActivationFunctionType.Exp, scale=-1.0,
                )
                E_tiles[(q, g)] = E

                # row part: sum the WHOLE band (j >= 8g) — with strict
                # stripes this is s fakes + diagonal + all j>i terms; the
                # column part supplies i<n. Fakes/diagonal-dup are constants
                # the host subtracts. g<3 on Pool add-trees (keeps the DVE
                # on absdiff); g=3 as a DVE reduce — the DVE is idle by then
                # and the Pool trees would serialize into the tail.
                oslice = orow_sb[:, (q * G + g) * HWL:(q * G + g + 1) * HWL]
                if g < 3 and TAIL_ENGINE == "pool":
                    tail_tree(E, 0, L, oslice)
                else:
                    Ev = E.rearrange("p (j hw) -> p hw j", j=L)
                    nc.vector.tensor_reduce(
                        out=oslice, in_=Ev, axis=mybir.AxisListType.X,
                        op=mybir.AluOpType.add,
                    )

            # emit any column parts whose band prerequisites are now done;
            # h<3 slices of ocol ship mid-stream, h=3 ships in the tail
            done_bands.add(g)
            for h in range(G):
                if ("col", h) not in done_bands and all(
                        gp in done_bands for gp in range(h + 1)):
                    col_part(h, h)
                    done_bands.add(("col", h))
            if all(("col", h) in done_bands for h in range(3)) \
                    and "ocol_dma" not in done_bands:
                nc.scalar.dma_start(out=ocol_d[:, 0:3 * GS * HWL],
                                    in_=ocol_sb[:, 0:3 * GS * HWL])
                done_bands.add("ocol_dma")

        # ship the g<3 row parts as soon as their trees finish; only the
        # tiny g3 slice (computed in the tail) goes in the final DMA
        orv_d = orow_d.rearrange("p (q g hw) -> p q g hw", q=Q, g=G)
        orv_s = orow_sb.rearrange("p (q g hw) -> p q g hw", q=Q, g=G)
        nc.sync.dma_start(out=orv_d[:, :, 0:3], in_=orv_s[:, :, 0:3])
        nc.sync.dma_start(out=orv_d[:, :, 3], in_=orv_s[:, :, 3])
        nc.scalar.dma_start(out=ocol_d[:, 3 * GS * HWL:],
                            in_=ocol_sb[:, 3 * GS * HWL:])


# --------------------------------------------------------------------------
# host side
# --------------------------------------------------------------------------

def prep_inputs(x, T):
    """Shared (core-independent) device inputs; xTg is sliced per core."""
    # xTg[f, fh, n, hw] = x[n, hw, fh*128+f]
    xTg = np.ascontiguousarray(
        x.reshape(N, HW, FH, 128).transpose(3, 2, 0, 1)
    ).astype(np.float16)
    # tw[f, (fh,q), b*8+c]
    tw = T.reshape(FH, 128, Q, 16, C).transpose(1, 0, 2, 3, 4)
    tw_in = np.ascontiguousarray(tw.reshape(128, FH * Q * 128)).astype(np.float16)
    ones_in = np.zeros((128, 8, 128), np.float16)
    for s in range(8):
        for b in range(16):
            ones_in[b * 8:(b + 1) * 8, s, 16 * s + b] = 1.0
    ones_in = ones_in.reshape(128, 1024)
    onescol_in = np.zeros((128, Q, 64), np.float16)
    for q in range(Q):
        for s in range(8):
            for b in range(16):
                onescol_in[16 * s + b, q, 16 * q + b] = 1.0
    onescol_in = onescol_in.reshape(128, Q * 64)
    return xTg, tw_in, ones_in, onescol_in


def core_in_map(xTg, tw_in, ones_in, onescol_in, k):
    xT = np.ascontiguousarray(
        xTg[:, :, :, k * HWL:(k + 1) * HWL]
    ).reshape(128, FH * N * HWL)
    return {"xT": xT, "tw": tw_in, "ones": ones_in, "onescol": onescol_in}


def gather_ob(core_outs):
    """core_outs: list of 8 dicts {orow:[128,512], ocol:[64,1024]} -> o_b."""
    # strict-triangle fakes: exp(0)=1 cells the device summed, removed here.
    # ocol self-stripes add (7-jj) fakes per column jj. orow full-band sums
    # add s fakes, and the diagonal is counted in both parts (+1).
    colfix = ((7 - np.arange(GS)).astype(np.float32))[None, :, None, None]
    rowfix = (np.arange(8) + 1.0)[None, :, None, None, None].astype(np.float32)
    obs = []
    for res in core_outs:
        c = res["ocol"].astype(np.float32).reshape(Q, 16, G, GS, HWL)
        # [q, b2, h, jj, hw] -> [n=(h,jj), hw, b=(q,b2)]
        ob = np.ascontiguousarray(
            c.transpose(2, 3, 4, 0, 1)).reshape(G, GS, HWL, B)
        ob -= colfix
        ob = ob.reshape(N, HWL, B)
        r = res["orow"].astype(np.float32).reshape(8, 16, Q, G, HWL)
        # [s, b2, q, g, hw] -> [n=(g,s), hw, b=(q,b2)]
        rr = r.transpose(3, 0, 4, 2, 1).copy()      # [g, s, hw, q, b2]
        rr -= rowfix
        ob += rr.reshape(N, HWL, B)
        obs.append(ob)
    # core k owns hw slice k -> stack on hw axis
    full = np.stack(obs, axis=1).reshape(N, HW, B)
    return full.reshape(N, 16, 16, B)


_CACHED = {}


def _get_program(reps=1, loop=None):
    key = ("nc", reps, loop)
    if key in _CACHED:
        return _CACHED[key]
    from contextlib import ExitStack
    import concourse.bacc as bacc
    import concourse.mybir as mybir
    import concourse.tile as tile

    nc = bacc.Bacc("TRN2", target_bir_lowering=False, debug=False,
                   num_devices=CORES)
    f16, f32 = mybir.dt.float16, mybir.dt.float32
    ins = {
        "xT": nc.dram_tensor("xT", [128, FH * N * HWL], f16,
                             kind="ExternalInput").ap(),
        "tw": nc.dram_tensor("tw", [128, FH * Q * 128], f16,
                             kind="ExternalInput").ap(),
        "ones": nc.dram_tensor("ones", [128, 1024], f16,
                               kind="ExternalInput").ap(),
        "onescol": nc.dram_tensor("onescol", [128, Q * 64], f16,
                                  kind="ExternalInput").ap(),
    }
    outs = {
        "orow": nc.dram_tensor("orow", [128, Q * G * HWL], f32,
                               kind="ExternalOutput").ap(),
        "ocol": nc.dram_tensor("ocol", [64, G * GS * HWL], f32,
                               kind="ExternalOutput").ap(),
    }
    with tile.TileContext(nc) as tc:
        if loop:
            with ExitStack() as ctx:
                pools = make_pools(tc, ctx)
                with tc.For_i(0, loop, 1,
                              hint_engines=(mybir.EngineType.PE,
                                            mybir.EngineType.DVE)):
                    build_body(tc, outs, ins, pools=pools)
        else:
            for r in range(reps):
                build_body(tc, outs, ins, rep=r)
    nc.compile()
    _CACHED[key] = nc
    return nc


def kernel(x, T):
    x = np.asarray(x, dtype=np.float32)
    T = np.asarray(T, dtype=np.float32)
    from concourse.bass_utils import run_bass_kernel_spmd

    nc = _get_program()
    xTg, tw_in, ones_in, onescol_in = prep_inputs(x, T)
    in_maps = [core_in_map(xTg, tw_in, ones_in, onescol_in, k)
               for k in range(CORES)]
    res = run_bass_kernel_spmd(nc, in_maps, core_ids=list(range(CORES)))
    ob = gather_ob(res.results)
    return np.concatenate([x, ob], axis=3)
